# revision 8
# baseline (speedup 1.0000x reference)
"""Trainium2 Bass kernel for SingleDGC (GCNConv + per-batch pairwise-distance
adjacency), data-parallel over 8 NeuronCores.

kernel(X, edge_index, W, b) -> (Xo [512,32,256] f32, adj_mask [512,32,32] bool)

Design notes:
- Each core owns 2048 contiguous target nodes (64 graphs). Edges are
  partitioned by target window on the host, sorted by (window, source-half),
  padded per 128-target window, and gathered per-edge with dma_gather.
- Critical path is the Q7 SWDGE descriptor generation (~8ns/edge). All DVE
  work is shaped to avoid the DVE<->GpSimd shared SBUF port (PSUM-sourced
  in0, broadcast SBUF in1 on the dedicated read port, no AP-scalar reads in
  hot ops) so it runs concurrently with descriptor generation.
- XW = X @ W runs in fp32; Y rows are scaled by dinv[src] and split into
  bf16 hi+lo (relative error ~2^-17), so each 128-edge chunk scatters with a
  single bf16 [128x128] x [128x512] matmul (hi and lo side by side in PSUM).
- The bias is folded into the accumulation as a K=1 matmul of
  sqrt(deg)[tgt] x (b_hi|b_lo), so the dinv post-scale reproduces +b.
- Stage 2 squared distances use an augmented matmul: -2*x_i.x_j via two
  K=128 bf-free fp32 matmuls, +sq_i and +sq_j via K=1 rank-1 matmuls.
"""
import numpy as np

B, A, D = 512, 32, 256
N = B * A                 # 16384 nodes
NCORES = 8
TPC = N // NCORES         # 2048 targets per core
WIN = 128                 # targets per PSUM window
NW = TPC // WIN           # 16 windows per core
NWG = N // WIN            # 128 global windows
EPS = 1e-5
SPLITS = [0, 2048, 8192, N]   # source ranges for XW/gather overlap

_compiled = {}            # CPs tuple -> nc


def _build(*CPs):
    import concourse.bacc as bacc
    import concourse.tile as tile
    from concourse import mybir
    from concourse.masks import make_identity
    import contextlib

    dt = mybir.dt
    P = 128
    CPW = sum(CPs)                    # chunks per window
    NCH = NW * CPW                    # chunks per core
    EPW = CPW * P

    nc = bacc.Bacc("TRN2", target_bir_lowering=False, debug=False,
                   num_devices=NCORES)

    XT = nc.dram_tensor("XT", [D, N], dt.float32, kind="ExternalInput").ap()
    Wm = nc.dram_tensor("Wm", [D, D], dt.float32, kind="ExternalInput").ap()
    dinv_full_t = nc.dram_tensor("dinv_full_t", [P, N // P], dt.float32,
                                 kind="ExternalInput").ap()
    dinv_t = nc.dram_tensor("dinv_t", [P, NW], dt.float32, kind="ExternalInput").ap()
    srcw = nc.dram_tensor("srcw", [P, NW * EPW // 16], dt.int16,
                          kind="ExternalInput").ap()
    tgtr = nc.dram_tensor("tgtr", [P, NCH], dt.float32, kind="ExternalInput").ap()
    sqdeg = nc.dram_tensor("sqdeg", [1, TPC], dt.bfloat16, kind="ExternalInput").ap()
    b_hilo = nc.dram_tensor("b_hilo", [1, 2 * D], dt.bfloat16,
                            kind="ExternalInput").ap()

    Xo_out = nc.dram_tensor("Xo_out", [TPC, D], dt.float32, kind="ExternalOutput").ap()
    adj_out = nc.dram_tensor("adj_out", [TPC, A], dt.uint8, kind="ExternalOutput").ap()

    with tile.TileContext(nc) as tc:
        with contextlib.ExitStack() as ctx:
            const = ctx.enter_context(tc.tile_pool(name="const", bufs=1))
            xt_pool = ctx.enter_context(tc.tile_pool(name="xt", bufs=3))
            y_pool = ctx.enter_context(tc.tile_pool(name="y", bufs=3))
            dram = ctx.enter_context(tc.tile_pool(name="dram", bufs=1, space="DRAM"))
            gath_pool = ctx.enter_context(tc.tile_pool(name="gath", bufs=3))
            m_pool = ctx.enter_context(tc.tile_pool(name="m", bufs=4))
            s2_pool = ctx.enter_context(tc.tile_pool(name="s2", bufs=2))
            xo_res = ctx.enter_context(tc.tile_pool(name="xores", bufs=1))
            # PSUM: mix(2) + pagg(2) + piota(1) + ptp(2) + psq(1) = 8 banks
            mix = ctx.enter_context(tc.tile_pool(name="mix", bufs=2, space="PSUM"))
            psum_agg = ctx.enter_context(tc.tile_pool(name="pagg", bufs=2, space="PSUM"))
            piota = ctx.enter_context(tc.tile_pool(name="piota", bufs=1, space="PSUM"))
            psum_tp = ctx.enter_context(tc.tile_pool(name="ptp", bufs=2, space="PSUM"))
            psum_sq = ctx.enter_context(tc.tile_pool(name="psq", bufs=1, space="PSUM"))

            # ---- constants
            w0 = const.tile([P, D], dt.float32)
            nc.sync.dma_start(w0[:], Wm[0:P, :])
            w1 = const.tile([P, D], dt.float32)
            nc.sync.dma_start(w1[:], Wm[P:D, :])
            t_dft = const.tile([P, N // P], dt.float32)
            nc.sync.dma_start(t_dft[:], dinv_full_t[:])
            t_dt = const.tile([P, NW], dt.float32)
            nc.sync.dma_start(t_dt[:], dinv_t[:])
            t_srcw = const.tile([P, NW * EPW // 16], dt.int16)
            nc.sync.dma_start(t_srcw[:], srcw[:])
            t_tgtr = const.tile([P, NCH], dt.float32)
            nc.sync.dma_start(t_tgtr[:], tgtr[:])
            t_sqdeg = const.tile([1, TPC], dt.bfloat16)
            nc.sync.dma_start(t_sqdeg[:], sqdeg[:])
            t_bhl = const.tile([1, 2 * D], dt.bfloat16)
            nc.sync.dma_start(t_bhl[:], b_hilo[:])
            iota_i = const.tile([P, P], dt.int32)
            nc.gpsimd.iota(iota_i[:], pattern=[[1, P]], base=0, channel_multiplier=0)
            iota_ps = piota.tile([P, P], dt.float32)
            nc.vector.tensor_copy(iota_ps[:], iota_i[:])
            ident = const.tile([P, P], dt.float32)
            make_identity(nc, ident[:])
            ones_col = const.tile([P, 1], dt.float32)
            nc.vector.memset(ones_col[:], 1.0)

            Y_dram = dram.tile([N, 2 * D], dt.bfloat16)   # [hi(256) | lo(256)]

            # ---- phase 1: Y = (X @ W) * dinv[row], split to bf16 hi/lo
            for bt in range(N // 512):
                xt0 = xt_pool.tile([P, 512], dt.float32, tag="xt0")
                nc.sync.dma_start(xt0[:], XT[0:P, 512 * bt:512 * (bt + 1)])
                xt1 = xt_pool.tile([P, 512], dt.float32, tag="xt1")
                nc.sync.dma_start(xt1[:], XT[P:D, 512 * bt:512 * (bt + 1)])
                for j in range(4):
                    t = 4 * bt + j
                    pxw = mix.tile([P, 2 * D], dt.float32, tag="mix")
                    nc.tensor.matmul(out=pxw[:, 0:D], lhsT=xt0[:, P * j:P * (j + 1)],
                                     rhs=w0[:], start=True, stop=False)
                    nc.tensor.matmul(out=pxw[:, 0:D], lhsT=xt1[:, P * j:P * (j + 1)],
                                     rhs=w1[:], start=False, stop=True)
                    dbc = t_dft[:, t:t + 1].to_broadcast([P, D])
                    # y (f32, exact) staged in the same PSUM bank
                    nc.vector.tensor_tensor(out=pxw[:, D:2 * D], in0=pxw[:, 0:D],
                                            in1=dbc, op=mybir.AluOpType.mult)
                    ys = y_pool.tile([P, 2 * D], dt.bfloat16, tag="ys")
                    nc.vector.tensor_copy(ys[:, 0:D], pxw[:, D:2 * D])
                    nc.vector.tensor_tensor(out=ys[:, D:2 * D], in0=pxw[:, D:2 * D],
                                            in1=ys[:, 0:D],
                                            op=mybir.AluOpType.subtract)
                    nc.sync.dma_start(Y_dram[P * t:P * (t + 1), :], ys[:])

            # ---- phase 2: aggregation per window
            groups = []
            acc = 0
            for gi_, cp in enumerate(CPs):
                groups.append((acc, acc + cp,
                               Y_dram[SPLITS[gi_]:SPLITS[gi_ + 1], :]))
                acc += cp
            xo_sb = xo_res.tile([P, NW, D], dt.float32)
            for w in range(NW):
                pagg = psum_agg.tile([P, 2 * D], dt.float32)
                for (c0, c1, ysrc) in groups:
                    ng = c1 - c0
                    gt = gath_pool.tile([P, max(CPs), 2 * D], dt.bfloat16,
                                        tag="gt")
                    col0 = (w * CPW + c0) * 8
                    nc.gpsimd.dma_gather(gt[:, 0:ng, :], ysrc,
                                         t_srcw[:, col0:col0 + ng * 8],
                                         ng * P, ng * P, 2 * D,
                                         single_packet=False)
                    for c in range(c0, c1):
                        m = m_pool.tile([P, P], dt.bfloat16)
                        ci = w * CPW + c
                        nc.vector.tensor_tensor(
                            out=m[:], in0=iota_ps[:],
                            in1=t_tgtr[:, ci:ci + 1].to_broadcast([P, P]),
                            op=mybir.AluOpType.is_equal)
                        nc.tensor.matmul(out=pagg[:], lhsT=m[:], rhs=gt[:, c - c0, :],
                                         start=(c == 0), stop=False)
                # bias term: += sqdeg[tgt] (x) (b_hi | b_lo), K=1
                nc.tensor.matmul(out=pagg[:],
                                 lhsT=t_sqdeg[:, P * w:P * (w + 1)],
                                 rhs=t_bhl[:], start=False, stop=True)
                # xo = (hi + lo) * dinv[tgt]
                s1 = s2_pool.tile([P, D], dt.float32, tag="s1")
                nc.vector.tensor_copy(s1[:], pagg[:, 0:D])
                nc.vector.tensor_tensor(out=pagg[:, 0:D], in0=pagg[:, D:2 * D],
                                        in1=s1[:], op=mybir.AluOpType.add)
                nc.vector.tensor_tensor(
                    out=xo_sb[:, w, :], in0=pagg[:, 0:D],
                    in1=t_dt[:, w:w + 1].to_broadcast([P, D]),
                    op=mybir.AluOpType.mult)
                nc.sync.dma_start(Xo_out[P * w:P * (w + 1), :], xo_sb[:, w, :])

            # ---- phase 3: per-graph pairwise distance mask (4 graphs/window)
            for g in range(NW):
                xo_g = xo_sb[:, g, :]
                ptp0 = psum_tp.tile([P, P], dt.float32, tag="ptp")
                nc.tensor.transpose(out=ptp0[:], in_=xo_g[:, 0:P], identity=ident[:])
                xoT0 = s2_pool.tile([P, P], dt.float32, tag="xoT0")
                nc.vector.tensor_copy(xoT0[:], ptp0[:])
                ptp1 = psum_tp.tile([P, P], dt.float32, tag="ptp")
                nc.tensor.transpose(out=ptp1[:], in_=xo_g[:, P:D], identity=ident[:])
                xoT1 = s2_pool.tile([P, P], dt.float32, tag="xoT1")
                nc.vector.tensor_copy(xoT1[:], ptp1[:])

                # squares read the PSUM copy (keeps DVE off the shared port)
                sq0 = s2_pool.tile([P, P], dt.float32, tag="sq0")
                nc.vector.tensor_tensor(out=sq0[:], in0=ptp0[:], in1=xoT0[:],
                                        op=mybir.AluOpType.mult)
                n2xoT0 = s2_pool.tile([P, P], dt.float32, tag="n2xoT0")
                nc.vector.tensor_scalar(out=n2xoT0[:], in0=ptp0[:], scalar1=-2.0,
                                        scalar2=None, op0=mybir.AluOpType.mult)
                sq1 = s2_pool.tile([P, P], dt.float32, tag="sq1")
                nc.vector.tensor_tensor(out=sq1[:], in0=ptp1[:], in1=xoT1[:],
                                        op=mybir.AluOpType.mult)
                n2xoT1 = s2_pool.tile([P, P], dt.float32, tag="n2xoT1")
                nc.vector.tensor_scalar(out=n2xoT1[:], in0=ptp1[:], scalar1=-2.0,
                                        scalar2=None, op0=mybir.AluOpType.mult)

                psq = psum_sq.tile([1, P], dt.float32)
                nc.tensor.matmul(out=psq[:], lhsT=ones_col[:], rhs=sq0[:],
                                 start=True, stop=False)
                nc.tensor.matmul(out=psq[:], lhsT=ones_col[:], rhs=sq1[:],
                                 start=False, stop=True)
                sqrow = s2_pool.tile([1, P], dt.float32, tag="sqrow")
                nc.vector.tensor_copy(sqrow[:], psq[:])
                ones_row = s2_pool.tile([1, P], dt.float32, tag="ones_row")
                nc.vector.memset(ones_row[:], 1.0)

                adj_g = s2_pool.tile([P, A], dt.uint8, tag="adj")
                for i in range(4):
                    sl = slice(A * i, A * (i + 1))
                    psc = mix.tile([A, A], dt.float32, tag="mix")
                    nc.tensor.matmul(out=psc[:], lhsT=n2xoT0[:, sl], rhs=xoT0[:, sl],
                                     start=True, stop=False)
                    nc.tensor.matmul(out=psc[:], lhsT=n2xoT1[:, sl], rhs=xoT1[:, sl],
                                     start=False, stop=False)
                    nc.tensor.matmul(out=psc[:], lhsT=sqrow[:, sl], rhs=ones_row[:, sl],
                                     start=False, stop=False)
                    nc.tensor.matmul(out=psc[:], lhsT=ones_row[:, sl], rhs=sqrow[:, sl],
                                     start=False, stop=True)
                    smin = s2_pool.tile([A, 1], dt.float32, tag="smin")
                    nc.vector.tensor_reduce(smin[:], psc[:], axis=mybir.AxisListType.X,
                                            op=mybir.AluOpType.min)
                    smax_ps = mix.tile([A, 1], dt.float32, tag="mix")
                    nc.vector.tensor_reduce(smax_ps[:], psc[:],
                                            axis=mybir.AxisListType.X,
                                            op=mybir.AluOpType.max)
                    # thr = 0.5*smin + 0.5*EPS + 0.5*smax
                    thr = s2_pool.tile([A, 1], dt.float32, tag="thr")
                    nc.vector.tensor_scalar(out=thr[:], in0=smin[:],
                                            scalar1=0.5, scalar2=0.5 * EPS,
                                            op0=mybir.AluOpType.mult,
                                            op1=mybir.AluOpType.add)
                    nc.vector.scalar_tensor_tensor(out=thr[:], in0=smax_ps[:],
                                                   scalar=0.5, in1=thr[:],
                                                   op0=mybir.AluOpType.mult,
                                                   op1=mybir.AluOpType.add)
                    nc.vector.tensor_tensor(out=adj_g[A * i:A * (i + 1), :],
                                            in0=psc[:],
                                            in1=thr[:, :1].to_broadcast([A, A]),
                                            op=mybir.AluOpType.is_gt)
                nc.sync.dma_start(adj_out[P * g:P * (g + 1), :], adj_g[:])

    nc.compile()
    return nc


def _prep(X, edge_index, W, b):
    """Host-side sharding/layout prep (index-topology work only)."""
    import ml_dtypes
    Xf = np.ascontiguousarray(X, np.float32).reshape(N, D)
    XT = np.ascontiguousarray(Xf.T)                      # [D, N]
    src = edge_index[0].astype(np.int64)
    tgt = edge_index[1].astype(np.int64)
    deg = (np.bincount(tgt, minlength=N) + 1).astype(np.float64)
    dinv = (1.0 / np.sqrt(deg)).astype(np.float32)
    sqdeg_full = np.sqrt(deg).astype(np.float32)

    loop = np.arange(N, dtype=np.int64)
    src_all = np.concatenate([src, loop])
    tgt_all = np.concatenate([tgt, loop])

    # sort by (window, src-group) so chunks split cleanly at the SPLITS
    NG_ = len(SPLITS) - 1
    win = tgt_all // WIN
    grp = np.searchsorted(np.array(SPLITS[1:]), src_all, side="right").astype(np.int64)
    order = np.lexsort((src_all, grp, win))
    s_src = src_all[order]
    s_tgt = tgt_all[order]
    s_win = win[order]
    s_grp = grp[order]

    wh = s_win * NG_ + s_grp
    counts = np.bincount(wh, minlength=NG_ * NWG)
    CPs = tuple(int(np.ceil(counts[q::NG_].max() / 128)) for q in range(NG_))
    CPW = sum(CPs)
    EPW = CPW * 128
    grp_off = np.concatenate([[0], np.cumsum(np.array(CPs) * 128)])[:-1]
    grp_base = np.array(SPLITS[:-1])

    wh_starts = np.cumsum(counts) - counts
    pos_in_grp = np.arange(len(s_tgt)) - wh_starts[wh]
    slot = s_win * EPW + grp_off[s_grp] + pos_in_grp

    src_pad = np.zeros(NWG * EPW, np.int16)
    rel_pad = np.full(NWG * EPW, -1.0, np.float32)
    src_pad[slot] = (s_src - grp_base[s_grp]).astype(np.int16)
    rel_pad[slot] = (s_tgt - s_win * WIN).astype(np.float32)
    src_pad = src_pad.reshape(NWG, EPW)
    rel_pad = rel_pad.reshape(NWG, EPW)

    dinv_full_t = np.ascontiguousarray(dinv.reshape(N // 128, 128).T)
    Wc = np.ascontiguousarray(W, np.float32)
    bf = np.asarray(b, np.float32).reshape(1, D)
    b_hi = bf.astype(ml_dtypes.bfloat16)
    b_lo = (bf - b_hi.astype(np.float32)).astype(ml_dtypes.bfloat16)
    b_hilo = np.concatenate([b_hi, b_lo], 1)             # [1, 512]

    in_maps = []
    for c in range(NCORES):
        flat_src = src_pad[c * NW:(c + 1) * NW].reshape(-1)      # [NW*EPW]
        flat_rel = rel_pad[c * NW:(c + 1) * NW].reshape(-1)
        srcw = np.tile(np.ascontiguousarray(flat_src.reshape(-1, 16).T), (8, 1))
        tgtr = np.ascontiguousarray(flat_rel.reshape(NW * CPW, 128).T)
        dinv_t = np.ascontiguousarray(
            dinv[c * TPC:(c + 1) * TPC].reshape(NW, 128).T)
        sqdeg = sqdeg_full[c * TPC:(c + 1) * TPC].reshape(1, TPC) \
            .astype(ml_dtypes.bfloat16)
        in_maps.append(dict(XT=XT, Wm=Wc, b_hilo=b_hilo,
                            dinv_full_t=dinv_full_t, dinv_t=dinv_t,
                            srcw=srcw, tgtr=tgtr, sqdeg=sqdeg))
    return CPs, in_maps


def _run(X, edge_index, W, b):
    from concourse.bass_utils import run_bass_kernel_spmd
    key, in_maps = _prep(X, edge_index, W, b)
    if key not in _compiled:
        _compiled[key] = _build(*key)
    nc = _compiled[key]
    res = run_bass_kernel_spmd(nc, in_maps, core_ids=list(range(NCORES)))
    Xo = np.concatenate([r["Xo_out"] for r in res.results], 0).reshape(B, A, D)
    adj = np.concatenate([r["adj_out"] for r in res.results], 0)
    adj = adj.reshape(B, A, A).astype(bool)
    return Xo, adj


def kernel(X, edge_index, W, b):
    return _run(np.asarray(X), np.asarray(edge_index), np.asarray(W), np.asarray(b))


# revision 9
# speedup vs baseline: 1.0236x; 1.0236x over previous
"""Trainium2 Bass kernel for SingleDGC (GCNConv + per-batch pairwise-distance
adjacency), data-parallel over 8 NeuronCores.

kernel(X, edge_index, W, b) -> (Xo [512,32,256] f32, adj_mask [512,32,32] bool)

Design notes:
- Each core owns 2048 contiguous target nodes (64 graphs). Edges are
  partitioned by target window on the host, sorted by (window, source-half),
  padded per 128-target window, and gathered per-edge with dma_gather.
- Critical path is the Q7 SWDGE descriptor generation (~8ns/edge). All DVE
  work is shaped to avoid the DVE<->GpSimd shared SBUF port (PSUM-sourced
  in0, broadcast SBUF in1 on the dedicated read port, no AP-scalar reads in
  hot ops) so it runs concurrently with descriptor generation.
- XW = X @ W runs in fp32; Y rows are scaled by dinv[src] and split into
  bf16 hi+lo (relative error ~2^-17), so each 128-edge chunk scatters with a
  single bf16 [128x128] x [128x512] matmul (hi and lo side by side in PSUM).
- The bias is folded into the accumulation as a K=1 matmul of
  sqrt(deg)[tgt] x (b_hi|b_lo), so the dinv post-scale reproduces +b.
- Stage 2 squared distances use an augmented matmul: -2*x_i.x_j via two
  K=128 bf-free fp32 matmuls, +sq_i and +sq_j via K=1 rank-1 matmuls.
"""
import numpy as np

B, A, D = 512, 32, 256
N = B * A                 # 16384 nodes
NCORES = 8
TPC = N // NCORES         # 2048 targets per core
WIN = 128                 # targets per PSUM window
NW = TPC // WIN           # 16 windows per core
NWG = N // WIN            # 128 global windows
EPS = 1e-5
HALF = N // 2             # source split point for XW/gather overlap

_compiled = {}            # (CPA, CPB) -> nc


def _build(CPA, CPB):
    import concourse.bacc as bacc
    import concourse.tile as tile
    from concourse import mybir
    from concourse.masks import make_identity
    import contextlib

    dt = mybir.dt
    P = 128
    CPW = CPA + CPB                   # chunks per window
    NCH = NW * CPW                    # chunks per core
    EPW = CPW * P

    nc = bacc.Bacc("TRN2", target_bir_lowering=False, debug=False,
                   num_devices=NCORES)

    XT = nc.dram_tensor("XT", [D, N], dt.float32, kind="ExternalInput").ap()
    Wm = nc.dram_tensor("Wm", [D, D], dt.float32, kind="ExternalInput").ap()
    dinv_full_t = nc.dram_tensor("dinv_full_t", [P, N // P], dt.float32,
                                 kind="ExternalInput").ap()
    dinv_t = nc.dram_tensor("dinv_t", [P, NW], dt.float32, kind="ExternalInput").ap()
    srcw = nc.dram_tensor("srcw", [P, NW * EPW // 16], dt.int16,
                          kind="ExternalInput").ap()
    tgtr = nc.dram_tensor("tgtr", [P, NCH], dt.float32, kind="ExternalInput").ap()
    sqdeg = nc.dram_tensor("sqdeg", [1, TPC], dt.bfloat16, kind="ExternalInput").ap()
    b_hilo = nc.dram_tensor("b_hilo", [1, 2 * D], dt.bfloat16,
                            kind="ExternalInput").ap()

    Xo_out = nc.dram_tensor("Xo_out", [TPC, D], dt.float32, kind="ExternalOutput").ap()
    adj_out = nc.dram_tensor("adj_out", [TPC, A], dt.uint8, kind="ExternalOutput").ap()

    with tile.TileContext(nc) as tc:
        with contextlib.ExitStack() as ctx:
            const = ctx.enter_context(tc.tile_pool(name="const", bufs=1))
            xt_pool = ctx.enter_context(tc.tile_pool(name="xt", bufs=3))
            y_pool = ctx.enter_context(tc.tile_pool(name="y", bufs=3))
            dram = ctx.enter_context(tc.tile_pool(name="dram", bufs=1, space="DRAM"))
            gath_pool = ctx.enter_context(tc.tile_pool(name="gath", bufs=3))
            m_pool = ctx.enter_context(tc.tile_pool(name="m", bufs=4))
            s2_pool = ctx.enter_context(tc.tile_pool(name="s2", bufs=2))
            xo_res = ctx.enter_context(tc.tile_pool(name="xores", bufs=1))
            # PSUM: mix(2) + pagg(2) + piota(1) + ptp(2) + psq(1) = 8 banks
            mix = ctx.enter_context(tc.tile_pool(name="mix", bufs=2, space="PSUM"))
            psum_agg = ctx.enter_context(tc.tile_pool(name="pagg", bufs=2, space="PSUM"))
            piota = ctx.enter_context(tc.tile_pool(name="piota", bufs=1, space="PSUM"))
            psum_tp = ctx.enter_context(tc.tile_pool(name="ptp", bufs=2, space="PSUM"))
            psum_sq = ctx.enter_context(tc.tile_pool(name="psq", bufs=1, space="PSUM"))

            # ---- constants
            w0 = const.tile([P, D], dt.float32)
            nc.sync.dma_start(w0[:], Wm[0:P, :])
            w1 = const.tile([P, D], dt.float32)
            nc.sync.dma_start(w1[:], Wm[P:D, :])
            t_dft = const.tile([P, N // P], dt.float32)
            nc.sync.dma_start(t_dft[:], dinv_full_t[:])
            t_dt = const.tile([P, NW], dt.float32)
            nc.sync.dma_start(t_dt[:], dinv_t[:])
            t_srcw = const.tile([P, NW * EPW // 16], dt.int16)
            nc.sync.dma_start(t_srcw[:], srcw[:])
            t_tgtr = const.tile([P, NCH], dt.float32)
            nc.sync.dma_start(t_tgtr[:], tgtr[:])
            t_sqdeg = const.tile([1, TPC], dt.bfloat16)
            nc.sync.dma_start(t_sqdeg[:], sqdeg[:])
            t_bhl = const.tile([1, 2 * D], dt.bfloat16)
            nc.sync.dma_start(t_bhl[:], b_hilo[:])
            iota_i = const.tile([P, P], dt.int32)
            nc.gpsimd.iota(iota_i[:], pattern=[[1, P]], base=0, channel_multiplier=0)
            iota_ps = piota.tile([P, P], dt.float32)
            nc.vector.tensor_copy(iota_ps[:], iota_i[:])
            ident = const.tile([P, P], dt.float32)
            make_identity(nc, ident[:])
            ones_col = const.tile([P, 1], dt.float32)
            nc.vector.memset(ones_col[:], 1.0)

            Y_dram = dram.tile([N, 2 * D], dt.bfloat16)   # [hi(256) | lo(256)]

            # ---- phase 1: Y = (X @ W) * dinv[row], split to bf16 hi/lo
            for bt in range(N // 512):
                xt0 = xt_pool.tile([P, 512], dt.float32, tag="xt0")
                nc.sync.dma_start(xt0[:], XT[0:P, 512 * bt:512 * (bt + 1)])
                xt1 = xt_pool.tile([P, 512], dt.float32, tag="xt1")
                nc.sync.dma_start(xt1[:], XT[P:D, 512 * bt:512 * (bt + 1)])
                for j in range(4):
                    t = 4 * bt + j
                    pxw = mix.tile([P, 2 * D], dt.float32, tag="mix")
                    nc.tensor.matmul(out=pxw[:, 0:D], lhsT=xt0[:, P * j:P * (j + 1)],
                                     rhs=w0[:], start=True, stop=False)
                    nc.tensor.matmul(out=pxw[:, 0:D], lhsT=xt1[:, P * j:P * (j + 1)],
                                     rhs=w1[:], start=False, stop=True)
                    dbc = t_dft[:, t:t + 1].to_broadcast([P, D])
                    # y (f32, exact) staged in the same PSUM bank
                    nc.vector.tensor_tensor(out=pxw[:, D:2 * D], in0=pxw[:, 0:D],
                                            in1=dbc, op=mybir.AluOpType.mult)
                    ys = y_pool.tile([P, 2 * D], dt.bfloat16, tag="ys")
                    nc.vector.tensor_copy(ys[:, 0:D], pxw[:, D:2 * D])
                    nc.vector.tensor_tensor(out=ys[:, D:2 * D], in0=pxw[:, D:2 * D],
                                            in1=ys[:, 0:D],
                                            op=mybir.AluOpType.subtract)
                    nc.sync.dma_start(Y_dram[P * t:P * (t + 1), :], ys[:])

            # ---- phase 2: aggregation per window
            Y_A = Y_dram[0:HALF, :]
            Y_B = Y_dram[HALF:N, :]
            xo_sb = xo_res.tile([P, NW, D], dt.float32)
            for w in range(NW):
                pagg = psum_agg.tile([P, 2 * D], dt.float32)
                for (c0, c1, ysrc) in ((0, CPA, Y_A), (CPA, CPW, Y_B)):
                    ng = c1 - c0
                    gt = gath_pool.tile([P, max(CPA, CPB), 2 * D], dt.bfloat16,
                                        tag="gt")
                    col0 = (w * CPW + c0) * 8
                    nc.gpsimd.dma_gather(gt[:, 0:ng, :], ysrc,
                                         t_srcw[:, col0:col0 + ng * 8],
                                         ng * P, ng * P, 2 * D,
                                         single_packet=False)
                    for c in range(c0, c1):
                        m = m_pool.tile([P, P], dt.bfloat16)
                        ci = w * CPW + c
                        nc.vector.tensor_tensor(
                            out=m[:], in0=iota_ps[:],
                            in1=t_tgtr[:, ci:ci + 1].to_broadcast([P, P]),
                            op=mybir.AluOpType.is_equal)
                        nc.tensor.matmul(out=pagg[:], lhsT=m[:], rhs=gt[:, c - c0, :],
                                         start=(c == 0), stop=False)
                # bias term: += sqdeg[tgt] (x) (b_hi | b_lo), K=1
                nc.tensor.matmul(out=pagg[:],
                                 lhsT=t_sqdeg[:, P * w:P * (w + 1)],
                                 rhs=t_bhl[:], start=False, stop=True)
                # xo = (hi + lo) * dinv[tgt]
                s1 = s2_pool.tile([P, D], dt.float32, tag="s1")
                nc.vector.tensor_copy(s1[:], pagg[:, 0:D])
                nc.vector.tensor_tensor(out=pagg[:, 0:D], in0=pagg[:, D:2 * D],
                                        in1=s1[:], op=mybir.AluOpType.add)
                nc.vector.tensor_tensor(
                    out=xo_sb[:, w, :], in0=pagg[:, 0:D],
                    in1=t_dt[:, w:w + 1].to_broadcast([P, D]),
                    op=mybir.AluOpType.mult)
                nc.sync.dma_start(Xo_out[P * w:P * (w + 1), :], xo_sb[:, w, :])

            # ---- phase 3: per-graph pairwise distance mask (4 graphs/window)
            for g in range(NW):
                xo_g = xo_sb[:, g, :]
                ptp0 = psum_tp.tile([P, P], dt.float32, tag="ptp")
                nc.tensor.transpose(out=ptp0[:], in_=xo_g[:, 0:P], identity=ident[:])
                xoT0 = s2_pool.tile([P, P], dt.float32, tag="xoT0")
                nc.vector.tensor_copy(xoT0[:], ptp0[:])
                ptp1 = psum_tp.tile([P, P], dt.float32, tag="ptp")
                nc.tensor.transpose(out=ptp1[:], in_=xo_g[:, P:D], identity=ident[:])
                xoT1 = s2_pool.tile([P, P], dt.float32, tag="xoT1")
                nc.vector.tensor_copy(xoT1[:], ptp1[:])

                # squares read the PSUM copy (keeps DVE off the shared port)
                sq0 = s2_pool.tile([P, P], dt.float32, tag="sq0")
                nc.vector.tensor_tensor(out=sq0[:], in0=ptp0[:], in1=xoT0[:],
                                        op=mybir.AluOpType.mult)
                n2xoT0 = s2_pool.tile([P, P], dt.float32, tag="n2xoT0")
                nc.vector.tensor_scalar(out=n2xoT0[:], in0=ptp0[:], scalar1=-2.0,
                                        scalar2=None, op0=mybir.AluOpType.mult)
                sq1 = s2_pool.tile([P, P], dt.float32, tag="sq1")
                nc.vector.tensor_tensor(out=sq1[:], in0=ptp1[:], in1=xoT1[:],
                                        op=mybir.AluOpType.mult)
                n2xoT1 = s2_pool.tile([P, P], dt.float32, tag="n2xoT1")
                nc.vector.tensor_scalar(out=n2xoT1[:], in0=ptp1[:], scalar1=-2.0,
                                        scalar2=None, op0=mybir.AluOpType.mult)

                psq = psum_sq.tile([1, P], dt.float32)
                nc.tensor.matmul(out=psq[:], lhsT=ones_col[:], rhs=sq0[:],
                                 start=True, stop=False)
                nc.tensor.matmul(out=psq[:], lhsT=ones_col[:], rhs=sq1[:],
                                 start=False, stop=True)
                sqrow = s2_pool.tile([1, P], dt.float32, tag="sqrow")
                nc.vector.tensor_copy(sqrow[:], psq[:])
                ones_row = s2_pool.tile([1, P], dt.float32, tag="ones_row")
                nc.vector.memset(ones_row[:], 1.0)

                adj_g = s2_pool.tile([P, A], dt.uint8, tag="adj")
                for i in range(4):
                    sl = slice(A * i, A * (i + 1))
                    psc = mix.tile([A, A], dt.float32, tag="mix")
                    nc.tensor.matmul(out=psc[:], lhsT=n2xoT0[:, sl], rhs=xoT0[:, sl],
                                     start=True, stop=False)
                    nc.tensor.matmul(out=psc[:], lhsT=n2xoT1[:, sl], rhs=xoT1[:, sl],
                                     start=False, stop=False)
                    nc.tensor.matmul(out=psc[:], lhsT=sqrow[:, sl], rhs=ones_row[:, sl],
                                     start=False, stop=False)
                    nc.tensor.matmul(out=psc[:], lhsT=ones_row[:, sl], rhs=sqrow[:, sl],
                                     start=False, stop=True)
                    smin = s2_pool.tile([A, 1], dt.float32, tag="smin")
                    nc.vector.tensor_reduce(smin[:], psc[:], axis=mybir.AxisListType.X,
                                            op=mybir.AluOpType.min)
                    smax_ps = mix.tile([A, 1], dt.float32, tag="mix")
                    nc.vector.tensor_reduce(smax_ps[:], psc[:],
                                            axis=mybir.AxisListType.X,
                                            op=mybir.AluOpType.max)
                    # thr = 0.5*smin + 0.5*EPS + 0.5*smax
                    thr = s2_pool.tile([A, 1], dt.float32, tag="thr")
                    nc.vector.tensor_scalar(out=thr[:], in0=smin[:],
                                            scalar1=0.5, scalar2=0.5 * EPS,
                                            op0=mybir.AluOpType.mult,
                                            op1=mybir.AluOpType.add)
                    nc.vector.scalar_tensor_tensor(out=thr[:], in0=smax_ps[:],
                                                   scalar=0.5, in1=thr[:],
                                                   op0=mybir.AluOpType.mult,
                                                   op1=mybir.AluOpType.add)
                    nc.vector.tensor_tensor(out=adj_g[A * i:A * (i + 1), :],
                                            in0=psc[:],
                                            in1=thr[:, :1].to_broadcast([A, A]),
                                            op=mybir.AluOpType.is_gt)
                nc.sync.dma_start(adj_out[P * g:P * (g + 1), :], adj_g[:])

    nc.compile()
    return nc


def _prep(X, edge_index, W, b):
    """Host-side sharding/layout prep (index-topology work only)."""
    import ml_dtypes
    Xf = np.ascontiguousarray(X, np.float32).reshape(N, D)
    XT = np.ascontiguousarray(Xf.T)                      # [D, N]
    src = edge_index[0].astype(np.int64)
    tgt = edge_index[1].astype(np.int64)
    deg = (np.bincount(tgt, minlength=N) + 1).astype(np.float64)
    dinv = (1.0 / np.sqrt(deg)).astype(np.float32)
    sqdeg_full = np.sqrt(deg).astype(np.float32)

    loop = np.arange(N, dtype=np.int64)
    src_all = np.concatenate([src, loop])
    tgt_all = np.concatenate([tgt, loop])

    # sort by (window, src-half) so each window's chunks split cleanly at HALF
    win = tgt_all // WIN
    half = (src_all >= HALF).astype(np.int64)
    order = np.lexsort((src_all, half, win))
    s_src = src_all[order]
    s_tgt = tgt_all[order]
    s_win = win[order]
    s_half = half[order]

    wh = s_win * 2 + s_half
    counts = np.bincount(wh, minlength=2 * NWG)
    CPA = int(np.ceil(counts[0::2].max() / 128))
    CPB = int(np.ceil(counts[1::2].max() / 128))
    CPW = CPA + CPB
    EPW = CPW * 128

    wh_starts = np.cumsum(counts) - counts
    pos_in_grp = np.arange(len(s_tgt)) - wh_starts[wh]
    slot = s_win * EPW + s_half * (CPA * 128) + pos_in_grp

    src_pad = np.zeros(NWG * EPW, np.int16)
    rel_pad = np.full(NWG * EPW, -1.0, np.float32)
    src_pad[slot] = (s_src - s_half * HALF).astype(np.int16)
    rel_pad[slot] = (s_tgt - s_win * WIN).astype(np.float32)
    src_pad = src_pad.reshape(NWG, EPW)
    rel_pad = rel_pad.reshape(NWG, EPW)

    dinv_full_t = np.ascontiguousarray(dinv.reshape(N // 128, 128).T)
    Wc = np.ascontiguousarray(W, np.float32)
    bf = np.asarray(b, np.float32).reshape(1, D)
    b_hi = bf.astype(ml_dtypes.bfloat16)
    b_lo = (bf - b_hi.astype(np.float32)).astype(ml_dtypes.bfloat16)
    b_hilo = np.concatenate([b_hi, b_lo], 1)             # [1, 512]

    in_maps = []
    for c in range(NCORES):
        flat_src = src_pad[c * NW:(c + 1) * NW].reshape(-1)      # [NW*EPW]
        flat_rel = rel_pad[c * NW:(c + 1) * NW].reshape(-1)
        srcw = np.tile(np.ascontiguousarray(flat_src.reshape(-1, 16).T), (8, 1))
        tgtr = np.ascontiguousarray(flat_rel.reshape(NW * CPW, 128).T)
        dinv_t = np.ascontiguousarray(
            dinv[c * TPC:(c + 1) * TPC].reshape(NW, 128).T)
        sqdeg = sqdeg_full[c * TPC:(c + 1) * TPC].reshape(1, TPC) \
            .astype(ml_dtypes.bfloat16)
        in_maps.append(dict(XT=XT, Wm=Wc, b_hilo=b_hilo,
                            dinv_full_t=dinv_full_t, dinv_t=dinv_t,
                            srcw=srcw, tgtr=tgtr, sqdeg=sqdeg))
    return (CPA, CPB), in_maps


def _run(X, edge_index, W, b):
    from concourse.bass_utils import run_bass_kernel_spmd
    key, in_maps = _prep(X, edge_index, W, b)
    if key not in _compiled:
        _compiled[key] = _build(*key)
    nc = _compiled[key]
    res = run_bass_kernel_spmd(nc, in_maps, core_ids=list(range(NCORES)))
    Xo = np.concatenate([r["Xo_out"] for r in res.results], 0).reshape(B, A, D)
    adj = np.concatenate([r["adj_out"] for r in res.results], 0)
    adj = adj.reshape(B, A, A).astype(bool)
    return Xo, adj


def kernel(X, edge_index, W, b):
    return _run(np.asarray(X), np.asarray(edge_index), np.asarray(W), np.asarray(b))


# revision 10
# speedup vs baseline: 1.2776x; 1.2481x over previous
"""Trainium2 Bass kernel for SingleDGC (GCNConv + per-batch pairwise-distance
adjacency), data-parallel over 8 NeuronCores.

kernel(X, edge_index, W, b) -> (Xo [512,32,256] f32, adj_mask [512,32,32] bool)

Design notes:
- Each core owns 2048 contiguous target nodes (64 graphs). Edges are
  partitioned by target window on the host, sorted by (window, source-half),
  padded per 128-target window, and gathered per-edge with dma_gather.
- Critical path is the Q7 SWDGE descriptor generation (~8ns/edge). All DVE
  work is shaped to avoid the DVE<->GpSimd shared SBUF port (PSUM-sourced
  in0, broadcast SBUF in1 on the dedicated read port, no AP-scalar reads in
  hot ops) so it runs concurrently with descriptor generation.
- XW = X @ W runs in fp32; Y rows are scaled by dinv[src] and split into
  bf16 hi+lo (relative error ~2^-17), so each 128-edge chunk scatters with a
  single bf16 [128x128] x [128x512] matmul (hi and lo side by side in PSUM).
- The bias is folded into the accumulation as a K=1 matmul of
  sqrt(deg)[tgt] x (b_hi|b_lo), so the dinv post-scale reproduces +b.
- Stage 2 squared distances use an augmented matmul: -2*x_i.x_j via two
  K=128 bf-free fp32 matmuls, +sq_i and +sq_j via K=1 rank-1 matmuls.
"""
import numpy as np

B, A, D = 512, 32, 256
N = B * A                 # 16384 nodes
NCORES = 8
TPC = N // NCORES         # 2048 targets per core
WIN = 128                 # targets per PSUM window
NW = TPC // WIN           # 16 windows per core
NWG = N // WIN            # 128 global windows
EPS = 1e-5
HALF = N // 2             # source split point for XW/gather overlap

_compiled = {}            # (CPA, CPB) -> nc


def _build(CPA, CPB):
    import concourse.bacc as bacc
    import concourse.tile as tile
    from concourse import mybir
    from concourse.masks import make_identity
    import contextlib

    dt = mybir.dt
    P = 128
    CPW = CPA + CPB                   # chunks per window
    NCH = NW * CPW                    # chunks per core
    EPW = CPW * P

    nc = bacc.Bacc("TRN2", target_bir_lowering=False, debug=False,
                   num_devices=NCORES, num_swdge_queues=2)

    XT = nc.dram_tensor("XT", [D, N], dt.float32, kind="ExternalInput").ap()
    Wm = nc.dram_tensor("Wm", [D, D], dt.float32, kind="ExternalInput").ap()
    dinv_full_t = nc.dram_tensor("dinv_full_t", [P, N // P], dt.float32,
                                 kind="ExternalInput").ap()
    dinv_t = nc.dram_tensor("dinv_t", [P, NW], dt.float32, kind="ExternalInput").ap()
    srcw = nc.dram_tensor("srcw", [P, NW * EPW // 16], dt.int16,
                          kind="ExternalInput").ap()
    tgtr = nc.dram_tensor("tgtr", [P, NCH], dt.float32, kind="ExternalInput").ap()
    sqdeg = nc.dram_tensor("sqdeg", [1, TPC], dt.bfloat16, kind="ExternalInput").ap()
    b_hilo = nc.dram_tensor("b_hilo", [1, 2 * D], dt.bfloat16,
                            kind="ExternalInput").ap()

    Xo_out = nc.dram_tensor("Xo_out", [TPC, D], dt.float32, kind="ExternalOutput").ap()
    adj_out = nc.dram_tensor("adj_out", [TPC, A], dt.uint8, kind="ExternalOutput").ap()

    with tile.TileContext(nc) as tc:
        with contextlib.ExitStack() as ctx:
            const = ctx.enter_context(tc.tile_pool(name="const", bufs=1))
            xt_pool = ctx.enter_context(tc.tile_pool(name="xt", bufs=3))
            y_pool = ctx.enter_context(tc.tile_pool(name="y", bufs=3))
            dram = ctx.enter_context(tc.tile_pool(name="dram", bufs=1, space="DRAM"))
            gath_pool = ctx.enter_context(tc.tile_pool(name="gath", bufs=3))
            m_pool = ctx.enter_context(tc.tile_pool(name="m", bufs=4))
            s2_pool = ctx.enter_context(tc.tile_pool(name="s2", bufs=2))
            xo_res = ctx.enter_context(tc.tile_pool(name="xores", bufs=1))
            # PSUM: mix(2) + pagg(2) + piota(1) + ptp(2) + psq(1) = 8 banks
            mix = ctx.enter_context(tc.tile_pool(name="mix", bufs=2, space="PSUM"))
            psum_agg = ctx.enter_context(tc.tile_pool(name="pagg", bufs=2, space="PSUM"))
            piota = ctx.enter_context(tc.tile_pool(name="piota", bufs=1, space="PSUM"))
            psum_tp = ctx.enter_context(tc.tile_pool(name="ptp", bufs=2, space="PSUM"))
            psum_sq = ctx.enter_context(tc.tile_pool(name="psq", bufs=1, space="PSUM"))

            # ---- constants
            w0 = const.tile([P, D], dt.float32)
            nc.sync.dma_start(w0[:], Wm[0:P, :])
            w1 = const.tile([P, D], dt.float32)
            nc.sync.dma_start(w1[:], Wm[P:D, :])
            t_dft = const.tile([P, N // P], dt.float32)
            nc.sync.dma_start(t_dft[:], dinv_full_t[:])
            t_dt = const.tile([P, NW], dt.float32)
            nc.sync.dma_start(t_dt[:], dinv_t[:])
            t_srcw = const.tile([P, NW * EPW // 16], dt.int16)
            nc.sync.dma_start(t_srcw[:], srcw[:])
            t_tgtr = const.tile([P, NCH], dt.float32)
            nc.sync.dma_start(t_tgtr[:], tgtr[:])
            t_sqdeg = const.tile([1, TPC], dt.bfloat16)
            nc.sync.dma_start(t_sqdeg[:], sqdeg[:])
            t_bhl = const.tile([1, 2 * D], dt.bfloat16)
            nc.sync.dma_start(t_bhl[:], b_hilo[:])
            iota_i = const.tile([P, P], dt.int32)
            nc.gpsimd.iota(iota_i[:], pattern=[[1, P]], base=0, channel_multiplier=0)
            iota_ps = piota.tile([P, P], dt.float32)
            nc.vector.tensor_copy(iota_ps[:], iota_i[:])
            ident = const.tile([P, P], dt.float32)
            make_identity(nc, ident[:])
            ones_col = const.tile([P, 1], dt.float32)
            nc.vector.memset(ones_col[:], 1.0)

            Y_dram = dram.tile([N, 2 * D], dt.bfloat16)   # [hi(256) | lo(256)]

            # ---- phase 1: Y = (X @ W) * dinv[row], split to bf16 hi/lo
            for bt in range(N // 512):
                xt0 = xt_pool.tile([P, 512], dt.float32, tag="xt0")
                nc.sync.dma_start(xt0[:], XT[0:P, 512 * bt:512 * (bt + 1)])
                xt1 = xt_pool.tile([P, 512], dt.float32, tag="xt1")
                nc.sync.dma_start(xt1[:], XT[P:D, 512 * bt:512 * (bt + 1)])
                for j in range(4):
                    t = 4 * bt + j
                    pxw = mix.tile([P, 2 * D], dt.float32, tag="mix")
                    nc.tensor.matmul(out=pxw[:, 0:D], lhsT=xt0[:, P * j:P * (j + 1)],
                                     rhs=w0[:], start=True, stop=False)
                    nc.tensor.matmul(out=pxw[:, 0:D], lhsT=xt1[:, P * j:P * (j + 1)],
                                     rhs=w1[:], start=False, stop=True)
                    dbc = t_dft[:, t:t + 1].to_broadcast([P, D])
                    # y (f32, exact) staged in the same PSUM bank
                    nc.vector.tensor_tensor(out=pxw[:, D:2 * D], in0=pxw[:, 0:D],
                                            in1=dbc, op=mybir.AluOpType.mult)
                    ys = y_pool.tile([P, 2 * D], dt.bfloat16, tag="ys")
                    nc.vector.tensor_copy(ys[:, 0:D], pxw[:, D:2 * D])
                    nc.vector.tensor_tensor(out=ys[:, D:2 * D], in0=pxw[:, D:2 * D],
                                            in1=ys[:, 0:D],
                                            op=mybir.AluOpType.subtract)
                    nc.sync.dma_start(Y_dram[P * t:P * (t + 1), :], ys[:])

            # ---- phase 2: aggregation per window
            Y_A = Y_dram[0:HALF, :]
            Y_B = Y_dram[HALF:N, :]
            xo_sb = xo_res.tile([P, NW, D], dt.float32)
            for w in range(NW):
                pagg = psum_agg.tile([P, 2 * D], dt.float32)
                for (c0, c1, ysrc) in ((0, CPA, Y_A), (CPA, CPW, Y_B)):
                    ng = c1 - c0
                    gt = gath_pool.tile([P, max(CPA, CPB), 2 * D], dt.bfloat16,
                                        tag="gt")
                    col0 = (w * CPW + c0) * 8
                    nc.gpsimd.dma_gather(gt[:, 0:ng, :], ysrc,
                                         t_srcw[:, col0:col0 + ng * 8],
                                         ng * P, ng * P, 2 * D,
                                         single_packet=False,
                                         queue_num=(w * 2 + (c0 != 0)) % 2)
                    for c in range(c0, c1):
                        m = m_pool.tile([P, P], dt.bfloat16)
                        ci = w * CPW + c
                        nc.vector.tensor_tensor(
                            out=m[:], in0=iota_ps[:],
                            in1=t_tgtr[:, ci:ci + 1].to_broadcast([P, P]),
                            op=mybir.AluOpType.is_equal)
                        nc.tensor.matmul(out=pagg[:], lhsT=m[:], rhs=gt[:, c - c0, :],
                                         start=(c == 0), stop=False)
                # bias term: += sqdeg[tgt] (x) (b_hi | b_lo), K=1
                nc.tensor.matmul(out=pagg[:],
                                 lhsT=t_sqdeg[:, P * w:P * (w + 1)],
                                 rhs=t_bhl[:], start=False, stop=True)
                # xo = (hi + lo) * dinv[tgt]
                s1 = s2_pool.tile([P, D], dt.float32, tag="s1")
                nc.vector.tensor_copy(s1[:], pagg[:, 0:D])
                nc.vector.tensor_tensor(out=pagg[:, 0:D], in0=pagg[:, D:2 * D],
                                        in1=s1[:], op=mybir.AluOpType.add)
                nc.vector.tensor_tensor(
                    out=xo_sb[:, w, :], in0=pagg[:, 0:D],
                    in1=t_dt[:, w:w + 1].to_broadcast([P, D]),
                    op=mybir.AluOpType.mult)
                nc.sync.dma_start(Xo_out[P * w:P * (w + 1), :], xo_sb[:, w, :])

            # ---- phase 3: per-graph pairwise distance mask (4 graphs/window)
            for g in range(NW):
                xo_g = xo_sb[:, g, :]
                ptp0 = psum_tp.tile([P, P], dt.float32, tag="ptp")
                nc.tensor.transpose(out=ptp0[:], in_=xo_g[:, 0:P], identity=ident[:])
                xoT0 = s2_pool.tile([P, P], dt.float32, tag="xoT0")
                nc.vector.tensor_copy(xoT0[:], ptp0[:])
                ptp1 = psum_tp.tile([P, P], dt.float32, tag="ptp")
                nc.tensor.transpose(out=ptp1[:], in_=xo_g[:, P:D], identity=ident[:])
                xoT1 = s2_pool.tile([P, P], dt.float32, tag="xoT1")
                nc.vector.tensor_copy(xoT1[:], ptp1[:])

                # squares read the PSUM copy (keeps DVE off the shared port)
                sq0 = s2_pool.tile([P, P], dt.float32, tag="sq0")
                nc.vector.tensor_tensor(out=sq0[:], in0=ptp0[:], in1=xoT0[:],
                                        op=mybir.AluOpType.mult)
                n2xoT0 = s2_pool.tile([P, P], dt.float32, tag="n2xoT0")
                nc.vector.tensor_scalar(out=n2xoT0[:], in0=ptp0[:], scalar1=-2.0,
                                        scalar2=None, op0=mybir.AluOpType.mult)
                sq1 = s2_pool.tile([P, P], dt.float32, tag="sq1")
                nc.vector.tensor_tensor(out=sq1[:], in0=ptp1[:], in1=xoT1[:],
                                        op=mybir.AluOpType.mult)
                n2xoT1 = s2_pool.tile([P, P], dt.float32, tag="n2xoT1")
                nc.vector.tensor_scalar(out=n2xoT1[:], in0=ptp1[:], scalar1=-2.0,
                                        scalar2=None, op0=mybir.AluOpType.mult)

                psq = psum_sq.tile([1, P], dt.float32)
                nc.tensor.matmul(out=psq[:], lhsT=ones_col[:], rhs=sq0[:],
                                 start=True, stop=False)
                nc.tensor.matmul(out=psq[:], lhsT=ones_col[:], rhs=sq1[:],
                                 start=False, stop=True)
                sqrow = s2_pool.tile([1, P], dt.float32, tag="sqrow")
                nc.vector.tensor_copy(sqrow[:], psq[:])
                ones_row = s2_pool.tile([1, P], dt.float32, tag="ones_row")
                nc.vector.memset(ones_row[:], 1.0)

                adj_g = s2_pool.tile([P, A], dt.uint8, tag="adj")
                for i in range(4):
                    sl = slice(A * i, A * (i + 1))
                    psc = mix.tile([A, A], dt.float32, tag="mix")
                    nc.tensor.matmul(out=psc[:], lhsT=n2xoT0[:, sl], rhs=xoT0[:, sl],
                                     start=True, stop=False)
                    nc.tensor.matmul(out=psc[:], lhsT=n2xoT1[:, sl], rhs=xoT1[:, sl],
                                     start=False, stop=False)
                    nc.tensor.matmul(out=psc[:], lhsT=sqrow[:, sl], rhs=ones_row[:, sl],
                                     start=False, stop=False)
                    nc.tensor.matmul(out=psc[:], lhsT=ones_row[:, sl], rhs=sqrow[:, sl],
                                     start=False, stop=True)
                    smin = s2_pool.tile([A, 1], dt.float32, tag="smin")
                    nc.vector.tensor_reduce(smin[:], psc[:], axis=mybir.AxisListType.X,
                                            op=mybir.AluOpType.min)
                    smax_ps = mix.tile([A, 1], dt.float32, tag="mix")
                    nc.vector.tensor_reduce(smax_ps[:], psc[:],
                                            axis=mybir.AxisListType.X,
                                            op=mybir.AluOpType.max)
                    # thr = 0.5*smin + 0.5*EPS + 0.5*smax
                    thr = s2_pool.tile([A, 1], dt.float32, tag="thr")
                    nc.vector.tensor_scalar(out=thr[:], in0=smin[:],
                                            scalar1=0.5, scalar2=0.5 * EPS,
                                            op0=mybir.AluOpType.mult,
                                            op1=mybir.AluOpType.add)
                    nc.vector.scalar_tensor_tensor(out=thr[:], in0=smax_ps[:],
                                                   scalar=0.5, in1=thr[:],
                                                   op0=mybir.AluOpType.mult,
                                                   op1=mybir.AluOpType.add)
                    nc.vector.tensor_tensor(out=adj_g[A * i:A * (i + 1), :],
                                            in0=psc[:],
                                            in1=thr[:, :1].to_broadcast([A, A]),
                                            op=mybir.AluOpType.is_gt)
                nc.sync.dma_start(adj_out[P * g:P * (g + 1), :], adj_g[:])

    nc.compile()
    return nc


def _prep(X, edge_index, W, b):
    """Host-side sharding/layout prep (index-topology work only)."""
    import ml_dtypes
    Xf = np.ascontiguousarray(X, np.float32).reshape(N, D)
    XT = np.ascontiguousarray(Xf.T)                      # [D, N]
    src = edge_index[0].astype(np.int64)
    tgt = edge_index[1].astype(np.int64)
    deg = (np.bincount(tgt, minlength=N) + 1).astype(np.float64)
    dinv = (1.0 / np.sqrt(deg)).astype(np.float32)
    sqdeg_full = np.sqrt(deg).astype(np.float32)

    loop = np.arange(N, dtype=np.int64)
    src_all = np.concatenate([src, loop])
    tgt_all = np.concatenate([tgt, loop])

    # sort by (window, src-half) so each window's chunks split cleanly at HALF
    win = tgt_all // WIN
    half = (src_all >= HALF).astype(np.int64)
    order = np.lexsort((src_all, half, win))
    s_src = src_all[order]
    s_tgt = tgt_all[order]
    s_win = win[order]
    s_half = half[order]

    wh = s_win * 2 + s_half
    counts = np.bincount(wh, minlength=2 * NWG)
    CPA = int(np.ceil(counts[0::2].max() / 128))
    CPB = int(np.ceil(counts[1::2].max() / 128))
    CPW = CPA + CPB
    EPW = CPW * 128

    wh_starts = np.cumsum(counts) - counts
    pos_in_grp = np.arange(len(s_tgt)) - wh_starts[wh]
    slot = s_win * EPW + s_half * (CPA * 128) + pos_in_grp

    src_pad = np.zeros(NWG * EPW, np.int16)
    rel_pad = np.full(NWG * EPW, -1.0, np.float32)
    src_pad[slot] = (s_src - s_half * HALF).astype(np.int16)
    rel_pad[slot] = (s_tgt - s_win * WIN).astype(np.float32)
    src_pad = src_pad.reshape(NWG, EPW)
    rel_pad = rel_pad.reshape(NWG, EPW)

    dinv_full_t = np.ascontiguousarray(dinv.reshape(N // 128, 128).T)
    Wc = np.ascontiguousarray(W, np.float32)
    bf = np.asarray(b, np.float32).reshape(1, D)
    b_hi = bf.astype(ml_dtypes.bfloat16)
    b_lo = (bf - b_hi.astype(np.float32)).astype(ml_dtypes.bfloat16)
    b_hilo = np.concatenate([b_hi, b_lo], 1)             # [1, 512]

    in_maps = []
    for c in range(NCORES):
        flat_src = src_pad[c * NW:(c + 1) * NW].reshape(-1)      # [NW*EPW]
        flat_rel = rel_pad[c * NW:(c + 1) * NW].reshape(-1)
        srcw = np.tile(np.ascontiguousarray(flat_src.reshape(-1, 16).T), (8, 1))
        tgtr = np.ascontiguousarray(flat_rel.reshape(NW * CPW, 128).T)
        dinv_t = np.ascontiguousarray(
            dinv[c * TPC:(c + 1) * TPC].reshape(NW, 128).T)
        sqdeg = sqdeg_full[c * TPC:(c + 1) * TPC].reshape(1, TPC) \
            .astype(ml_dtypes.bfloat16)
        in_maps.append(dict(XT=XT, Wm=Wc, b_hilo=b_hilo,
                            dinv_full_t=dinv_full_t, dinv_t=dinv_t,
                            srcw=srcw, tgtr=tgtr, sqdeg=sqdeg))
    return (CPA, CPB), in_maps


def _run(X, edge_index, W, b):
    from concourse.bass_utils import run_bass_kernel_spmd
    key, in_maps = _prep(X, edge_index, W, b)
    if key not in _compiled:
        _compiled[key] = _build(*key)
    nc = _compiled[key]
    res = run_bass_kernel_spmd(nc, in_maps, core_ids=list(range(NCORES)))
    Xo = np.concatenate([r["Xo_out"] for r in res.results], 0).reshape(B, A, D)
    adj = np.concatenate([r["adj_out"] for r in res.results], 0)
    adj = adj.reshape(B, A, A).astype(bool)
    return Xo, adj


def kernel(X, edge_index, W, b):
    return _run(np.asarray(X), np.asarray(edge_index), np.asarray(W), np.asarray(b))
